# revision 1
# baseline (speedup 1.0000x reference)
"""Cross-attention kernel for Trainium2 (8 NeuronCores, Bass/Tile).

Sharding: core c handles batch b = c//2 and head-group hg = c%2 (8 of 16
heads).  Each core computes, for its (b, hg):
  - q/k/v projections (weights column-sliced per head group, fp32r)
  - per-head masked softmax attention (scores in PSUM, mask folded in via an
    identity-matmul accumulation, exp+row-sums on the scalar engine)
  - att-mean partial (sum over its 8 heads of att/16) -> host adds the two
    head-group halves
  - y partial = att @ v @ Wp[:, hg-cols].T -> host adds halves + bias
Numerics: fp32r (fp32 with 12 mantissa bits RNE-truncated; full-rate on the
PE at free-dim >= 256) for projections/scores, fp16 for attention
probabilities and the AV path.
"""

import os
import sys

sys.path.insert(0, "/opt/trn_rl_repo")

import numpy as np

import concourse.bass as bass
import concourse.tile as tile
from concourse import mybir
import concourse.bass_utils as bu

# ---------------------------------------------------------------- constants
B, T, TE, C = 4, 2048, 1024, 1024
H = 16          # total heads
HG = 8          # heads per group (per core)
D = 64          # head dim
KT = 9          # contraction tiles for cin = 1024 (+1 bias row, padded to 1152)
CIN_PAD = KT * 128
EXPB = -2.0     # constant exp bias (cancels in softmax, guards fp16 overflow)
NEG = -60000.0  # additive mask value (exp underflows to exactly 0)
N_CORES = 8

f32 = mybir.dt.float32
f32r = mybir.dt.float32r
f16 = mybir.dt.float16


def _rne12(x: np.ndarray) -> np.ndarray:
    """Round fp32 to fp32r (zero low 12 mantissa bits, round-nearest-even)."""
    b = np.ascontiguousarray(x, dtype=np.float32).view(np.uint32)
    half = np.uint32(1 << 11)
    lsb = (b >> np.uint32(12)) & np.uint32(1)
    out = (b + half - np.uint32(1) + lsb) & ~np.uint32((1 << 12) - 1)
    return out.view(np.float32)


def _split_waits(nc, max_waits=1):
    """walrus in this container accepts at most one sync-wait command per
    instruction; hoist extra waits onto preceding same-engine NoOps."""
    import bass_rust

    ctr = 0
    for f in nc.m.functions:
        for blk in f.blocks:
            il = list(blk.instructions)
            out = []
            changed = False
            for inst in il:
                si = inst.sync_info
                if si is not None and si.on_wait and len(si.on_wait) > max_waits:
                    waits = list(si.on_wait)
                    for w in waits[:-max_waits]:
                        ctr += 1
                        nop = mybir.InstNoOp(name=f"waitsplit_{ctr}", ins=[], outs=[])
                        nop.engine = inst.engine
                        nop.sync_info = bass_rust.SyncInfo(on_wait=[w], on_update=[])
                        out.append(nop)
                    inst.sync_info = bass_rust.SyncInfo(
                        on_wait=waits[-max_waits:],
                        on_update=list(si.on_update) if si.on_update else [],
                    )
                    changed = True
                out.append(inst)
            if changed:
                blk.instructions = out


def _build_program():
    nc = bass.Bass("TRN2", target_bir_lowering=False, debug=False)

    xta_d = nc.declare_dram_parameter("xta", [CIN_PAD, T], f32r, isOutput=False)
    eta_d = nc.declare_dram_parameter("eta", [CIN_PAD, TE], f32r, isOutput=False)
    wq_d = nc.declare_dram_parameter("wqta", [CIN_PAD, 512], f32r, isOutput=False)
    wk_d = nc.declare_dram_parameter("wkta", [CIN_PAD, 512], f32r, isOutput=False)
    wv_d = nc.declare_dram_parameter("wvta", [CIN_PAD, 512], f32r, isOutput=False)
    wp_d = nc.declare_dram_parameter("wpt", [512, C], f32r, isOutput=False)
    mneg_d = nc.declare_dram_parameter("mneg", [T, TE], f16, isOutput=False)
    idh_d = nc.declare_dram_parameter("identh", [128, 128], f16, isOutput=False)
    id32_d = nc.declare_dram_parameter("ident32", [128, 128], f32, isOutput=False)
    y_d = nc.declare_dram_parameter("ypart", [T, C], f32, isOutput=True)
    a_d = nc.declare_dram_parameter("apart", [T, TE], f32, isOutput=True)

    xta_r = xta_d.rearrange("(kt p) n -> p kt n", p=128)
    eta_r = eta_d.rearrange("(kt p) n -> p kt n", p=128)
    wq_r = wq_d.rearrange("(kt p) n -> p kt n", p=128)
    wk_r = wk_d.rearrange("(kt p) n -> p kt n", p=128)
    wv_r = wv_d.rearrange("(kt p) n -> p kt n", p=128)
    wp_r = wp_d.rearrange("(kt p) n -> p kt n", p=128)

    with tile.TileContext(nc) as tc:
        with tc.tile_pool(name="persist", bufs=1) as persist:
            qT = persist.tile([128, 4, T], f32r, tag="qT")
            kT = persist.tile([128, 4, TE], f32r, tag="kT")
            vsb = persist.tile([128, 8, 512], f16, tag="vsb")
            wp = persist.tile([128, 4, C], f32r, tag="wp")
            idh = persist.tile([128, 128], f16, tag="idh")
            id32 = persist.tile([128, 128], f32, tag="id32")
            eb = persist.tile([128, 1], f32, tag="eb")

            nc.scalar.dma_start(out=idh, in_=idh_d[:, :])
            nc.scalar.dma_start(out=id32, in_=id32_d[:, :])
            nc.vector.memset(eb, EXPB)
            nc.scalar.dma_start(out=wp, in_=wp_r[:, :, :])

            # ---------------- stage A: projections ----------------
            psA_ctx = tc.tile_pool(name="psA", bufs=2, space="PSUM")
            psA = psA_ctx.__enter__()
            with tc.tile_pool(name="wqpool", bufs=1) as wqpool:
                wq = wqpool.tile([128, KT, 512], f32r, tag="wq")
                nc.scalar.dma_start(out=wq, in_=wq_r[:, :, :])
                with tc.tile_pool(name="xpool", bufs=2) as xpool:
                    for Tc in range(4):
                        tsl = slice(Tc * 512, (Tc + 1) * 512)
                        xt = xpool.tile([128, KT, 512], f32r, tag="xt")
                        nc.scalar.dma_start(out=xt, in_=xta_r[:, :, tsl])
                        for pt in range(4):
                            ps = psA.tile([128, 512], f32, tag="psA")
                            for kt in range(KT):
                                nc.tensor.matmul(
                                    ps[:, :],
                                    wq[:, kt, pt * 128:(pt + 1) * 128],
                                    xt[:, kt, :],
                                    start=(kt == 0), stop=(kt == KT - 1),
                                )
                            nc.scalar.copy(qT[:, pt, tsl], ps[:, :])

            with tc.tile_pool(name="wkvpool", bufs=1) as wkvpool:
                wk = wkvpool.tile([128, KT, 512], f32r, tag="wk")
                wv = wkvpool.tile([128, KT, 512], f32r, tag="wv")
                nc.scalar.dma_start(out=wk, in_=wk_r[:, :, :])
                nc.scalar.dma_start(out=wv, in_=wv_r[:, :, :])
                with tc.tile_pool(name="epool", bufs=2) as epool:
                    for sh in range(2):
                        ssl = slice(sh * 512, (sh + 1) * 512)
                        et = epool.tile([128, KT, 512], f32r, tag="et")
                        nc.scalar.dma_start(out=et, in_=eta_r[:, :, ssl])
                        for pt in range(4):
                            ps = psA.tile([128, 512], f32, tag="psA")
                            for kt in range(KT):
                                nc.tensor.matmul(
                                    ps[:, :],
                                    wk[:, kt, pt * 128:(pt + 1) * 128],
                                    et[:, kt, :],
                                    start=(kt == 0), stop=(kt == KT - 1),
                                )
                            nc.scalar.copy(kT[:, pt, ssl], ps[:, :])
                        for st4 in range(4):
                            ps = psA.tile([128, 512], f32, tag="psA")
                            for kt in range(KT):
                                nc.tensor.matmul(
                                    ps[:, :],
                                    et[:, kt, st4 * 128:(st4 + 1) * 128],
                                    wv[:, kt, :],
                                    start=(kt == 0), stop=(kt == KT - 1),
                                )
                            nc.scalar.copy(vsb[:, sh * 4 + st4, :], ps[:, :])
            psA_ctx.__exit__(None, None, None)

            # ---------------- stage B: attention ----------------
            with (
                tc.tile_pool(name="spool", bufs=2, space="PSUM") as spool,
                tc.tile_pool(name="ypool", bufs=2, space="PSUM") as ypool,
                tc.tile_pool(name="ppool", bufs=1, space="PSUM") as ppool,
                tc.tile_pool(name="rpool", bufs=1, space="PSUM") as rpool,
                tc.tile_pool(name="persistB", bufs=1) as persistB,
                tc.tile_pool(name="attEpool", bufs=2) as attEpool,
                tc.tile_pool(name="attTpool", bufs=3) as attTpool,
                tc.tile_pool(name="mpool", bufs=2) as mpool,
                tc.tile_pool(name="accpool", bufs=2) as accpool,
                tc.tile_pool(name="ytpool", bufs=2) as ytpool,
                tc.tile_pool(name="zpool", bufs=2) as zpool,
                tc.tile_pool(name="rsbpool", bufs=2) as rsbpool,
                tc.tile_pool(name="bpool", bufs=4) as bpool,
                tc.tile_pool(name="opool", bufs=2) as opool,
                tc.tile_pool(name="drpool", bufs=1, space="DRAM") as drpool,
            ):
                rT_dram = drpool.tile([HG, T], f32, tag="rT")

                def av_proj(tci, aT):
                    tsl = slice(tci * 128, (tci + 1) * 128)
                    # AV (col-packed head pairs) + per-row normalize
                    bc = bpool.tile([64, HG, 128], f32, tag="bc")
                    nc.sync.dma_start(
                        out=bc, in_=rT_dram[:, tsl].partition_broadcast(64)
                    )
                    yts = ytpool.tile([128, 4, 128], f32r, tag="yts")
                    for hp in range(4):
                        yps = ypool.tile([128, 128], f32, tag="yps")
                        for st in range(8):
                            for h2 in range(2):
                                h = hp * 2 + h2
                                nc.tensor.matmul(
                                    yps[h2 * 64:(h2 + 1) * 64, :],
                                    vsb[:, st, h * 64:(h + 1) * 64],
                                    aT[:, h * 8 + st, :],
                                    start=(st == 0), stop=(st == 7),
                                    tile_position=(0, h2 * 64),
                                )
                        for h2 in range(2):
                            h = hp * 2 + h2
                            nc.vector.tensor_mul(
                                yts[h2 * 64:(h2 + 1) * 64, hp, :],
                                yps[h2 * 64:(h2 + 1) * 64, :],
                                bc[:, h, :],
                            )
                    # output projection (partial over this head group's columns)
                    for ch in range(2):
                        csl = slice(ch * 512, (ch + 1) * 512)
                        pps = ppool.tile([128, 512], f32, tag="pps")
                        for kt in range(4):
                            nc.tensor.matmul(
                                pps[:, :], yts[:, kt, :], wp[:, kt, csl],
                                start=(kt == 0), stop=(kt == 3),
                            )
                        oc = opool.tile([128, 512], f32, tag="oc")
                        nc.scalar.copy(oc[:, :], pps[:, :])
                        nc.gpsimd.dma_start(out=y_d[tsl, csl], in_=oc[:, :])

                pending = None
                for tci in range(16):
                    tsl = slice(tci * 128, (tci + 1) * 128)
                    mk = mpool.tile([128, TE], f16, tag="mk")
                    nc.gpsimd.dma_start(out=mk, in_=mneg_d[tsl, :])
                    aE = attEpool.tile([128, HG, TE], f16, tag="aE")
                    aT = attTpool.tile([128, HG * 8, 128], f16, tag="aT")
                    Zs = zpool.tile([128, HG], f32, tag="Zs")
                    for hp in range(4):
                        S0 = spool.tile([128, TE], f32, tag="S")
                        S1 = spool.tile([128, TE], f32, tag="S")
                        for sh in range(2):
                            ssl = slice(sh * 512, (sh + 1) * 512)
                            for h2, S in ((0, S0), (1, S1)):
                                hrow = slice(h2 * 64, (h2 + 1) * 64)
                                nc.tensor.matmul(
                                    S[:, ssl],
                                    qT[hrow, hp, tsl],
                                    kT[hrow, hp, ssl],
                                    start=True, stop=False,
                                    tile_position=(h2 * 64, 0),
                                )
                        for h2, S in ((0, S0), (1, S1)):
                            for sh in range(2):
                                ssl = slice(sh * 512, (sh + 1) * 512)
                                nc.tensor.matmul(
                                    S[:, ssl], idh[:, :], mk[:, ssl],
                                    start=False, stop=True,
                                )
                        for h2, S in ((0, S0), (1, S1)):
                            h = hp * 2 + h2
                            nc.scalar.activation(
                                aE[:, h, :], S[:, :],
                                mybir.ActivationFunctionType.Exp,
                                bias=eb[:, 0:1],
                                accum_out=Zs[:, h:h + 1],
                            )
                    rc = zpool.tile([128, HG], f32, tag="rc")
                    nc.vector.reciprocal(rc[:, :], Zs[:, :])
                    r16 = zpool.tile([128, HG], f32, tag="r16")
                    nc.vector.tensor_scalar_mul(r16[:, :], rc[:, :], 1.0 / H)
                    # att-mean partial (att/16 summed over this core's heads)
                    acc = accpool.tile([128, TE], f32, tag="acc")
                    nc.vector.tensor_scalar_mul(acc[:, :], aE[:, 0, :], r16[:, 0:1])
                    for h in range(1, HG):
                        nc.vector.scalar_tensor_tensor(
                            out=acc[:, :], in0=aE[:, h, :], scalar=r16[:, h:h + 1],
                            in1=acc[:, :],
                            op0=mybir.AluOpType.mult, op1=mybir.AluOpType.add,
                        )
                    nc.gpsimd.dma_start(out=a_d[tsl, :], in_=acc[:, :])
                    # 1/Z transposed [h, t] -> DRAM for later row-broadcast
                    rt_ps = rpool.tile([HG, 128], f32, tag="rt")
                    nc.tensor.transpose(rt_ps[:, :], rc[:, :], id32[:, :])
                    rt_sb = rsbpool.tile([HG, 128], f32, tag="rtsb")
                    nc.scalar.copy(rt_sb[:, :], rt_ps[:, :])
                    nc.scalar.dma_start(out=rT_dram[:, tsl], in_=rt_sb[:, :])
                    # transpose attention probs for the AV contraction:
                    # one 2 MB xbar transfer (spreads across all 16 SDMA slots)
                    nc.sync.dma_start_transpose(aT[:, :, :], aE[:, :, :])
                    # software pipeline: AV+proj run one chunk behind so the
                    # PE's in-order stream never stalls on exp/transpose
                    if pending is not None:
                        av_proj(*pending)
                    pending = (tci, aT)
                av_proj(*pending)

    _split_waits(nc)
    return nc


_PROGRAM = None


def _get_program():
    global _PROGRAM
    if _PROGRAM is None:
        _PROGRAM = _build_program()
    return _PROGRAM


def _host_inputs(x, encoder_output, mask, Wq, bq, Wk, bk, Wv, bv, Wp, bp):
    """Build the 8 per-core input maps."""
    x = np.asarray(x, np.float32)
    enc = np.asarray(encoder_output, np.float32)
    mask = np.asarray(mask)
    scale = 1.0 / np.sqrt(D)
    identh = np.eye(128, dtype=np.float16)
    id32 = np.eye(128, dtype=np.float32)

    in_maps = []
    for c in range(N_CORES):
        b, hg = c // 2, c % 2
        hsl = slice(hg * 512, (hg + 1) * 512)

        xta = np.zeros((CIN_PAD, T), np.float32)
        xta[:C] = x[b].T
        xta[C] = 1.0
        eta = np.zeros((CIN_PAD, TE), np.float32)
        eta[:C] = enc[b].T
        eta[C] = 1.0

        wqta = np.zeros((CIN_PAD, 512), np.float32)
        wqta[:C] = (np.asarray(Wq, np.float32)[hsl] * scale).T
        wqta[C] = np.asarray(bq, np.float32)[hsl] * scale
        wkta = np.zeros((CIN_PAD, 512), np.float32)
        wkta[:C] = np.asarray(Wk, np.float32)[hsl].T
        wkta[C] = np.asarray(bk, np.float32)[hsl]
        wvta = np.zeros((CIN_PAD, 512), np.float32)
        wvta[:C] = np.asarray(Wv, np.float32)[hsl].T
        wvta[C] = np.asarray(bv, np.float32)[hsl]
        wpt = np.ascontiguousarray(np.asarray(Wp, np.float32)[:, hsl].T)

        mneg = (mask[b].astype(np.float16)) * np.float16(NEG)

        in_maps.append({
            "xta": _rne12(xta),
            "eta": _rne12(eta),
            "wqta": _rne12(wqta),
            "wkta": _rne12(wkta),
            "wvta": _rne12(wvta),
            "wpt": _rne12(wpt),
            "mneg": mneg,
            "identh": identh,
            "ident32": id32,
        })
    return in_maps


def kernel(x, encoder_output, mask, Wq, bq, Wk, bk, Wv, bv, Wp, bp):
    nc = _get_program()
    in_maps = _host_inputs(x, encoder_output, mask, Wq, bq, Wk, bk, Wv, bv, Wp, bp)
    trace = bool(int(os.environ.get("KERNEL_TRACE", "0")))
    res = bu.run_bass_kernel_spmd(nc, in_maps, list(range(N_CORES)), trace=trace)
    if trace:
        kernel.last_exec_time_ns = res.exec_time_ns
        kernel.last_profile = res
    outs = res.results

    bp = np.asarray(bp, np.float32)
    y = np.empty((B, T, C), np.float32)
    am = np.empty((B, T, TE), np.float32)
    for b in range(B):
        y[b] = outs[2 * b]["ypart"] + outs[2 * b + 1]["ypart"] + bp
        am[b] = outs[2 * b]["apart"] + outs[2 * b + 1]["apart"]
    return (y, am)



# revision 7
# speedup vs baseline: 1.1695x; 1.1695x over previous
"""Cross-attention kernel for Trainium2 (8 NeuronCores, Bass/Tile).

Sharding: core c handles batch b = c//2 and head-group hg = c%2 (8 of 16
heads).  Each core computes, for its (b, hg):
  - q/k/v projections (weights column-sliced per head group, fp32r)
  - per-head attention: scores in PSUM (fp32r, h2-paired via tile_position),
    unmasked exp on the scalar engine, then a single DVE
    tensor_tensor_reduce per head that multiplies by the 0/1 mask in-place
    AND produces the masked row-sums Z (accum_out)
  - att-mean partial: f16 scalar_tensor_tensor chain (sum_h att_h/16),
    written out as f16; host adds the two head-group halves in f32
  - AV runs two chunks behind (so the 2 MB prob transpose, which takes
    ~10us spread over all 16 SDMA engines, is fully off the critical path),
    then y partial = att @ v @ Wp[:, hg-cols].T; host adds halves + bias
Engine budget per 128-row chunk (target ~10us steady state):
  PE ~7us (QK+AV+proj), ACT ~8.6us (8 exps), DVE ~5-9us (TTR+STT+yts),
  sync ~9.7us (prob transpose), gpsimd (DMA issue + PSUM copies).
"""

import os
import sys

sys.path.insert(0, "/opt/trn_rl_repo")

import numpy as np

import concourse.bass as bass
import concourse.tile as tile
from concourse import mybir
import concourse.bass_utils as bu

# ---------------------------------------------------------------- constants
B, T, TE, C = 4, 2048, 1024, 1024
H = 16          # total heads
HG = 8          # heads per group (per core)
D = 64          # head dim
KT = 9          # contraction tiles for cin = 1024 (+1 bias row, padded to 1152)
CIN_PAD = KT * 128
EXPB = -2.0     # constant exp bias (cancels in softmax, guards fp16 overflow)
N_CORES = 8
LAG = 2         # AV/proj pipeline lag in chunks

f32 = mybir.dt.float32
f32r = mybir.dt.float32r
f16 = mybir.dt.float16


def _rne12(x: np.ndarray) -> np.ndarray:
    """Round fp32 to fp32r (zero low 12 mantissa bits, round-nearest-even)."""
    b = np.ascontiguousarray(x, dtype=np.float32).view(np.uint32)
    half = np.uint32(1 << 11)
    lsb = (b >> np.uint32(12)) & np.uint32(1)
    out = (b + half - np.uint32(1) + lsb) & ~np.uint32((1 << 12) - 1)
    return out.view(np.float32)


def _split_waits(nc, max_waits=1):
    """walrus in this container accepts at most one sync-wait command per
    instruction; hoist extra waits onto preceding same-engine NoOps."""
    import bass_rust

    ctr = 0
    for f in nc.m.functions:
        for blk in f.blocks:
            il = list(blk.instructions)
            out = []
            changed = False
            for inst in il:
                si = inst.sync_info
                if si is not None and si.on_wait and len(si.on_wait) > max_waits:
                    waits = list(si.on_wait)
                    for w in waits[:-max_waits]:
                        ctr += 1
                        nop = mybir.InstNoOp(name=f"waitsplit_{ctr}", ins=[], outs=[])
                        nop.engine = inst.engine
                        nop.sync_info = bass_rust.SyncInfo(on_wait=[w], on_update=[])
                        out.append(nop)
                    inst.sync_info = bass_rust.SyncInfo(
                        on_wait=waits[-max_waits:],
                        on_update=list(si.on_update) if si.on_update else [],
                    )
                    changed = True
                out.append(inst)
            if changed:
                blk.instructions = out


def _build_program():
    nc = bass.Bass("TRN2", target_bir_lowering=False, debug=False)

    xta_d = nc.declare_dram_parameter("xta", [CIN_PAD, T], f32r, isOutput=False)
    eta_d = nc.declare_dram_parameter("eta", [CIN_PAD, TE], f32r, isOutput=False)
    wq_d = nc.declare_dram_parameter("wqta", [CIN_PAD, 512], f32r, isOutput=False)
    wk_d = nc.declare_dram_parameter("wkta", [CIN_PAD, 512], f32r, isOutput=False)
    wv_d = nc.declare_dram_parameter("wvta", [CIN_PAD, 512], f32r, isOutput=False)
    wp_d = nc.declare_dram_parameter("wpt", [512, C], f32r, isOutput=False)
    m01_d = nc.declare_dram_parameter("m01", [T, TE], f16, isOutput=False)
    id32_d = nc.declare_dram_parameter("ident32", [128, 128], f32, isOutput=False)
    y_d = nc.declare_dram_parameter("ypart", [T, C], f32, isOutput=True)
    a_d = nc.declare_dram_parameter("apart", [T, TE], f16, isOutput=True)

    xta_r = xta_d.rearrange("(kt p) n -> p kt n", p=128)
    eta_r = eta_d.rearrange("(kt p) n -> p kt n", p=128)
    wq_r = wq_d.rearrange("(kt p) n -> p kt n", p=128)
    wk_r = wk_d.rearrange("(kt p) n -> p kt n", p=128)
    wv_r = wv_d.rearrange("(kt p) n -> p kt n", p=128)
    wp_r = wp_d.rearrange("(kt p) n -> p kt n", p=128)

    with tile.TileContext(nc) as tc:
        with tc.tile_pool(name="persist", bufs=1) as persist:
            qT = persist.tile([128, 4, T], f32r, tag="qT")
            kT = persist.tile([128, 4, TE], f32r, tag="kT")
            vsb = persist.tile([128, 8, 512], f16, tag="vsb")
            wp = persist.tile([128, 4, C], f32r, tag="wp")
            id32 = persist.tile([128, 128], f32, tag="id32")
            eb = persist.tile([128, 1], f32, tag="eb")

            nc.scalar.dma_start(out=id32, in_=id32_d[:, :])
            nc.vector.memset(eb, EXPB)
            nc.gpsimd.dma_start(out=wp, in_=wp_r[:, :, :])

            # ---------------- stage A: projections ----------------
            psA_ctx = tc.tile_pool(name="psA", bufs=2, space="PSUM")
            psA = psA_ctx.__enter__()
            with tc.tile_pool(name="wqpool", bufs=1) as wqpool:
                wq = wqpool.tile([128, KT, 512], f32r, tag="wq")
                nc.scalar.dma_start(out=wq, in_=wq_r[:, :, :])
                with tc.tile_pool(name="xpool", bufs=2) as xpool:
                    for Tc in range(4):
                        tsl = slice(Tc * 512, (Tc + 1) * 512)
                        xt = xpool.tile([128, KT, 512], f32r, tag="xt")
                        nc.scalar.dma_start(out=xt, in_=xta_r[:, :, tsl])
                        for pt in range(4):
                            ps = psA.tile([128, 512], f32, tag="psA")
                            for kt in range(KT):
                                nc.tensor.matmul(
                                    ps[:, :],
                                    wq[:, kt, pt * 128:(pt + 1) * 128],
                                    xt[:, kt, :],
                                    start=(kt == 0), stop=(kt == KT - 1),
                                )
                            nc.scalar.copy(qT[:, pt, tsl], ps[:, :])

            with tc.tile_pool(name="wkvpool", bufs=1) as wkvpool:
                wk = wkvpool.tile([128, KT, 512], f32r, tag="wk")
                wv = wkvpool.tile([128, KT, 512], f32r, tag="wv")
                nc.scalar.dma_start(out=wk, in_=wk_r[:, :, :])
                nc.scalar.dma_start(out=wv, in_=wv_r[:, :, :])
                with tc.tile_pool(name="epool", bufs=2) as epool:
                    for sh in range(2):
                        ssl = slice(sh * 512, (sh + 1) * 512)
                        et = epool.tile([128, KT, 512], f32r, tag="et")
                        nc.scalar.dma_start(out=et, in_=eta_r[:, :, ssl])
                        for pt in range(4):
                            ps = psA.tile([128, 512], f32, tag="psA")
                            for kt in range(KT):
                                nc.tensor.matmul(
                                    ps[:, :],
                                    wk[:, kt, pt * 128:(pt + 1) * 128],
                                    et[:, kt, :],
                                    start=(kt == 0), stop=(kt == KT - 1),
                                )
                            nc.scalar.copy(kT[:, pt, ssl], ps[:, :])
                        for st4 in range(4):
                            ps = psA.tile([128, 512], f32, tag="psA")
                            for kt in range(KT):
                                nc.tensor.matmul(
                                    ps[:, :],
                                    et[:, kt, st4 * 128:(st4 + 1) * 128],
                                    wv[:, kt, :],
                                    start=(kt == 0), stop=(kt == KT - 1),
                                )
                            nc.scalar.copy(vsb[:, sh * 4 + st4, :], ps[:, :])
            psA_ctx.__exit__(None, None, None)

            # ---------------- stage B: attention ----------------
            with (
                tc.tile_pool(name="spool", bufs=2, space="PSUM") as spool,
                tc.tile_pool(name="ypool", bufs=1, space="PSUM") as ypool,
                tc.tile_pool(name="ppool", bufs=2, space="PSUM") as ppool,
                tc.tile_pool(name="rpool", bufs=1, space="PSUM") as rpool,
                tc.tile_pool(name="attEpool", bufs=2) as attEpool,
                tc.tile_pool(name="attTpool", bufs=3) as attTpool,
                tc.tile_pool(name="mpool", bufs=3) as mpool,
                tc.tile_pool(name="accpool", bufs=2) as accpool,
                tc.tile_pool(name="ytpool", bufs=2) as ytpool,
                tc.tile_pool(name="zpool", bufs=2) as zpool,
                tc.tile_pool(name="rsbpool", bufs=2) as rsbpool,
                tc.tile_pool(name="bpool", bufs=4) as bpool,
                tc.tile_pool(name="opool", bufs=2) as opool,
                tc.tile_pool(name="drpool", bufs=1, space="DRAM") as drpool,
            ):
                rT_dram = drpool.tile([HG, T], f32, tag="rT")

                # prefetch first masks
                mks = {}
                for pre in range(min(LAG, 16)):
                    mk = mpool.tile([128, TE], f16, tag="mk")
                    nc.gpsimd.dma_start(
                        out=mk, in_=m01_d[pre * 128:(pre + 1) * 128, :])
                    mks[pre] = mk

                def av_heads(st_ctx, h0, h1):
                    """AV for heads h0..h1-1 of a pending chunk (paired)."""
                    tci, aT, bc, yts = st_ctx
                    for hp in range(h0 // 2, h1 // 2):
                        yps = ypool.tile([128, 128], f32, tag="yps")
                        for st in range(8):
                            for h2 in range(2):
                                h = hp * 2 + h2
                                nc.tensor.matmul(
                                    yps[h2 * 64:(h2 + 1) * 64, :],
                                    vsb[:, st, h * 64:(h + 1) * 64],
                                    aT[:, h * 8 + st, :],
                                    start=(st == 0), stop=(st == 7),
                                    tile_position=(0, h2 * 64),
                                )
                        for h2 in range(2):
                            h = hp * 2 + h2
                            nc.vector.tensor_mul(
                                yts[h2 * 64:(h2 + 1) * 64, hp, :],
                                yps[h2 * 64:(h2 + 1) * 64, :],
                                bc[:, h, :],
                            )

                def proj_out(st_ctx):
                    tci, aT, bc, yts = st_ctx
                    tsl = slice(tci * 128, (tci + 1) * 128)
                    for ch in range(2):
                        csl = slice(ch * 512, (ch + 1) * 512)
                        pps = ppool.tile([128, 512], f32, tag="pps")
                        for kt in range(4):
                            nc.tensor.matmul(
                                pps[:, :], yts[:, kt, :], wp[:, kt, csl],
                                start=(kt == 0), stop=(kt == 3),
                            )
                        oc = opool.tile([128, 512], f32, tag="oc")
                        if ch == 0:
                            nc.scalar.copy(oc[:, :], pps[:, :])
                        else:
                            nc.vector.tensor_copy(oc[:, :], pps[:, :])
                        nc.gpsimd.dma_start(out=y_d[tsl, csl], in_=oc[:, :])

                pending = []
                for tci in range(16):
                    tsl = slice(tci * 128, (tci + 1) * 128)
                    mk = mks.pop(tci)
                    if tci + LAG < 16:
                        nxt = tci + LAG
                        mkn = mpool.tile([128, TE], f16, tag="mk")
                        nc.gpsimd.dma_start(
                            out=mkn, in_=m01_d[nxt * 128:(nxt + 1) * 128, :])
                        mks[nxt] = mkn

                    aE = attEpool.tile([128, HG, TE], f16, tag="aE")
                    Zs = zpool.tile([128, HG], f32, tag="Zs")
                    ready = pending[0] if len(pending) >= LAG else None

                    for hp in range(4):
                        # interleave lagged AV/proj chunks between QK groups so
                        # the PE never idles while ACT works through the exps
                        if ready is not None:
                            if hp == 1:
                                av_heads(ready, 0, 4)
                            elif hp == 2:
                                av_heads(ready, 4, 8)
                            elif hp == 3:
                                proj_out(ready)
                        S0 = spool.tile([128, TE], f32, tag="S")
                        S1 = spool.tile([128, TE], f32, tag="S")
                        for h2, S in ((0, S0), (1, S1)):
                            hrow = slice(h2 * 64, (h2 + 1) * 64)
                            for sh in range(2):
                                ssl = slice(sh * 512, (sh + 1) * 512)
                                nc.tensor.matmul(
                                    S[:, ssl],
                                    qT[hrow, hp, tsl],
                                    kT[hrow, hp, ssl],
                                    start=True, stop=True,
                                    tile_position=(h2 * 64, 0),
                                )
                        for h2, S in ((0, S0), (1, S1)):
                            h = hp * 2 + h2
                            nc.scalar.activation(
                                aE[:, h, :], S[:, :],
                                mybir.ActivationFunctionType.Exp,
                                bias=eb[:, 0:1],
                            )
                            # mask multiply (in-place) + masked row-sums
                            nc.vector.scalar_tensor_tensor(
                                out=aE[:, h, :], in0=aE[:, h, :], scalar=1.0,
                                in1=mk[:, :],
                                op0=mybir.AluOpType.mult,
                                op1=mybir.AluOpType.mult,
                                accum_out=Zs[:, h:h + 1],
                            )

                    rc = zpool.tile([128, HG], f32, tag="rc")
                    nc.vector.reciprocal(rc[:, :], Zs[:, :])
                    r16 = zpool.tile([128, HG], f32, tag="r16")
                    nc.vector.tensor_scalar_mul(r16[:, :], rc[:, :], 1.0 / H)
                    # 1/Z transposed [h, t] -> DRAM for later row-broadcast
                    rt_ps = rpool.tile([HG, 128], f32, tag="rt")
                    nc.tensor.transpose(rt_ps[:, :], rc[:, :], id32[:, :])
                    rt_sb = rsbpool.tile([HG, 128], f32, tag="rtsb")
                    nc.scalar.copy(rt_sb[:, :], rt_ps[:, :])
                    nc.gpsimd.dma_start(out=rT_dram[:, tsl], in_=rt_sb[:, :])
                    bc = bpool.tile([64, HG, 128], f32, tag="bc")
                    nc.sync.dma_start(
                        out=bc, in_=rT_dram[:, tsl].partition_broadcast(64)
                    )
                    # att-mean partial (att/16 summed over this core's heads)
                    acc = accpool.tile([128, TE], f16, tag="acc")
                    nc.vector.tensor_scalar_mul(acc[:, :], aE[:, 0, :], r16[:, 0:1])
                    for h in range(1, HG):
                        nc.vector.scalar_tensor_tensor(
                            out=acc[:, :], in0=aE[:, h, :], scalar=r16[:, h:h + 1],
                            in1=acc[:, :],
                            op0=mybir.AluOpType.mult, op1=mybir.AluOpType.add,
                        )
                    nc.gpsimd.dma_start(out=a_d[tsl, :], in_=acc[:, :])
                    # transpose masked probs for the AV contraction:
                    # one 2 MB xbar transfer (spreads across all 16 SDMA slots)
                    aT = attTpool.tile([128, HG * 8, 128], f16, tag="aT")
                    nc.sync.dma_start_transpose(aT[:, :, :], aE[:, :, :])

                    yts = ytpool.tile([128, 4, 128], f32r, tag="yts")
                    pending.append((tci, aT, bc, yts))
                    if ready is not None:
                        pending.pop(0)

                # drain the last LAG chunks
                for st_ctx in pending:
                    av_heads(st_ctx, 0, 4)
                    av_heads(st_ctx, 4, 8)
                    proj_out(st_ctx)

    _split_waits(nc)
    return nc


_PROGRAM = None


def _get_program():
    global _PROGRAM
    if _PROGRAM is None:
        _PROGRAM = _build_program()
    return _PROGRAM


def _host_inputs(x, encoder_output, mask, Wq, bq, Wk, bk, Wv, bv, Wp, bp):
    """Build the 8 per-core input maps."""
    x = np.asarray(x, np.float32)
    enc = np.asarray(encoder_output, np.float32)
    mask = np.asarray(mask)
    scale = 1.0 / np.sqrt(D)
    id32 = np.eye(128, dtype=np.float32)

    in_maps = []
    for c in range(N_CORES):
        b, hg = c // 2, c % 2
        hsl = slice(hg * 512, (hg + 1) * 512)

        xta = np.zeros((CIN_PAD, T), np.float32)
        xta[:C] = x[b].T
        xta[C] = 1.0
        eta = np.zeros((CIN_PAD, TE), np.float32)
        eta[:C] = enc[b].T
        eta[C] = 1.0

        wqta = np.zeros((CIN_PAD, 512), np.float32)
        wqta[:C] = (np.asarray(Wq, np.float32)[hsl] * scale).T
        wqta[C] = np.asarray(bq, np.float32)[hsl] * scale
        wkta = np.zeros((CIN_PAD, 512), np.float32)
        wkta[:C] = np.asarray(Wk, np.float32)[hsl].T
        wkta[C] = np.asarray(bk, np.float32)[hsl]
        wvta = np.zeros((CIN_PAD, 512), np.float32)
        wvta[:C] = np.asarray(Wv, np.float32)[hsl].T
        wvta[C] = np.asarray(bv, np.float32)[hsl]
        wpt = np.ascontiguousarray(np.asarray(Wp, np.float32)[:, hsl].T)

        m01 = (~mask[b]).astype(np.float16)

        in_maps.append({
            "xta": _rne12(xta),
            "eta": _rne12(eta),
            "wqta": _rne12(wqta),
            "wkta": _rne12(wkta),
            "wvta": _rne12(wvta),
            "wpt": _rne12(wpt),
            "m01": m01,
            "ident32": id32,
        })
    return in_maps


def kernel(x, encoder_output, mask, Wq, bq, Wk, bk, Wv, bv, Wp, bp):
    nc = _get_program()
    in_maps = _host_inputs(x, encoder_output, mask, Wq, bq, Wk, bk, Wv, bv, Wp, bp)
    trace = bool(int(os.environ.get("KERNEL_TRACE", "0")))
    res = bu.run_bass_kernel_spmd(nc, in_maps, list(range(N_CORES)), trace=trace)
    if trace:
        kernel.last_exec_time_ns = res.exec_time_ns
        kernel.last_profile = res
    outs = res.results

    bp = np.asarray(bp, np.float32)
    y = np.empty((B, T, C), np.float32)
    am = np.empty((B, T, TE), np.float32)
    for b in range(B):
        y[b] = outs[2 * b]["ypart"] + outs[2 * b + 1]["ypart"] + bp
        am[b] = (outs[2 * b]["apart"].astype(np.float32)
                 + outs[2 * b + 1]["apart"].astype(np.float32))
    return (y, am)
